# revision 16
# baseline (speedup 1.0000x reference)
"""CantorMultiheadFusion kernel for 8 Trainium2 NeuronCores.

Math: out = x + A @ x @ (W_in @ W_out) + b_out, where A is the (S,S) sparse
fusion matrix with A[s, routes[s,k]] += fusion_weights[s,k].

Strategy (per core): data-parallel over (batch b, seq quarter q); each core
computes 1024 output rows. The sparse gather-fuse runs as a dense matmul on
the PE array in transposed layout so the projection chains without any
on-device transposes:
  phase A: axT[d, s]  = sum_src x[src, d] * A^T[src, s]       (lhsT = x blocks)
  phase B: outT[d2, s] = sum_d Wc[d, d2] * axT[d, s] + (x^T + b_out)[d2, s]

The output is produced transposed ([D, rows] per core); the host reassembles
the (B, S, D) layout. On-device math is bf16 with fp32 PSUM accumulation; the
residual+bias tensor stays fp32. Host preprocessing is input repacking only:
densifying the routing tables into A^T, casting to bf16, transposing slices.
"""

import numpy as np
import ml_dtypes

B, S, D, K = 2, 4096, 512, 32
NCORES = 8
QROWS = S // 4  # rows per core = 1024
DBLK = D // 128  # 4
KBLK = S // 128  # 32

_bf16 = ml_dtypes.bfloat16

_cache = {}


def _build_module():
    import concourse.mybir as mybir
    import concourse.tile as tile
    from concourse import bacc

    f32 = mybir.dt.float32
    bf16 = mybir.dt.bfloat16

    nc = bacc.Bacc("TRN2", target_bir_lowering=True)

    xb = nc.dram_tensor("xb", [S, D], bf16, kind="ExternalInput")
    at = nc.dram_tensor("at", [S, QROWS], bf16, kind="ExternalInput")
    wc = nc.dram_tensor("wc", [D, D], bf16, kind="ExternalInput")
    xrb = nc.dram_tensor("xrb", [D, QROWS], f32, kind="ExternalInput")
    outT = nc.dram_tensor("outT", [D, QROWS], f32, kind="ExternalOutput")

    with tile.TileContext(nc) as tc:
        with (
            tc.tile_pool(name="const", bufs=1) as cpool,
            tc.tile_pool(name="work", bufs=3) as wpool,
            tc.tile_pool(name="psum", bufs=4, space="PSUM") as ppool,
        ):
            # --- streamed loads (xb/at first: phase A consumes them) ---------
            xb_sb = []  # x[b] row-block k: [128, D]
            at_sb = []  # A^T row-block k: [128, QROWS]
            for k in range(KBLK):
                t = cpool.tile([128, D], bf16, tag=f"xb{k}")
                nc.sync.dma_start(out=t, in_=xb[k * 128 : (k + 1) * 128, :])
                xb_sb.append(t)
                t = cpool.tile([128, QROWS], bf16, tag=f"at{k}")
                nc.scalar.dma_start(out=t, in_=at[k * 128 : (k + 1) * 128, :])
                at_sb.append(t)

            wc_sb = []  # Wc row-block d1: [128, D] (needed only in phase B)
            for d1 in range(DBLK):
                t = cpool.tile([128, D], bf16, tag=f"wc{d1}")
                nc.sync.dma_start(out=t, in_=wc[d1 * 128 : (d1 + 1) * 128, :])
                wc_sb.append(t)

            xrb_sb = []  # (x^T + b_out) block d2: [128, QROWS] fp32
            for d2 in range(DBLK):
                t = cpool.tile([128, QROWS], f32, tag=f"xrb{d2}")
                nc.sync.dma_start(out=t, in_=xrb[d2 * 128 : (d2 + 1) * 128, :])
                xrb_sb.append(t)

            # --- phase A: axT[d] = x-block-col-d ^T @ A^T --------------------
            # k outer / d inner: each at-tile is consumed 4x right after its
            # DMA lands, so the PE never waits on the A^T stream. d=0 is
            # front-loaded (d>0 lag by ~10 k-steps) so the PSUM->SBUF copy of
            # axT[0..2] and phase B's early chains hide under phase A.
            ps_a = [
                ppool.tile([128, QROWS], f32, tag="ps", name=f"ps_a{d}")
                for d in range(DBLK)
            ]

            def _mm_a(d, k):
                for h in range(2):
                    nc.tensor.matmul(
                        ps_a[d][:, h * 512 : (h + 1) * 512],
                        xb_sb[k][:, d * 128 : (d + 1) * 128],
                        at_sb[k][:, h * 512 : (h + 1) * 512],
                        start=(k == 0),
                        stop=(k == KBLK - 1),
                    )

            for k in range(KBLK):
                for d in range(DBLK):
                    _mm_a(d, k)
            axT = []
            for d in range(DBLK):
                t = wpool.tile([128, QROWS], bf16, tag=f"axT{d}")
                if d % 2 == 0:
                    nc.vector.tensor_copy(t, ps_a[d])
                else:
                    nc.scalar.activation(
                        t, ps_a[d], mybir.ActivationFunctionType.Copy
                    )
                axT.append(t)

            # --- phase B: outT[d2] = Wc-chain @ axT + (x^T + b_out) ----------
            for d2 in range(DBLK):
                ps_b = ppool.tile([128, QROWS], f32, tag="ps", name=f"ps_b{d2}")
                for d1 in range(DBLK):
                    for h in range(2):
                        nc.tensor.matmul(
                            ps_b[:, h * 512 : (h + 1) * 512],
                            wc_sb[d1][:, d2 * 128 : (d2 + 1) * 128],
                            axT[d1][:, h * 512 : (h + 1) * 512],
                            start=(d1 == 0),
                            stop=(d1 == DBLK - 1),
                        )
                o = wpool.tile([128, QROWS], f32, tag="osb", name=f"osb{d2}")
                nc.vector.tensor_tensor(o, ps_b, xrb_sb[d2], mybir.AluOpType.add)
                nc.sync.dma_start(out=outT[d2 * 128 : (d2 + 1) * 128, :], in_=o)

    nc.finalize()
    return nc


def _get_runner():
    """Compile once; return a callable(list_of_in_maps) -> list_of_out_dicts."""
    if "runner" in _cache:
        return _cache["runner"]

    import jax
    from jax.sharding import Mesh, PartitionSpec
    from jax.experimental.shard_map import shard_map
    from concourse import bass2jax
    import concourse.mybir as mybir

    bass2jax.install_neuronx_cc_hook()
    nc = _build_module()

    part_name = nc.partition_id_tensor.name if nc.partition_id_tensor else None
    in_names = []
    out_names = []
    out_avals = []
    for alloc in nc.m.functions[0].allocations:
        if not isinstance(alloc, bass2jax.mybir.MemoryLocationSet):
            continue
        name = alloc.memorylocations[0].name
        if alloc.kind == "ExternalInput":
            if name != part_name:
                in_names.append(name)
        elif alloc.kind == "ExternalOutput":
            out_names.append(name)
            out_avals.append(
                jax.core.ShapedArray(
                    tuple(alloc.tensor_shape), mybir.dt.np(alloc.dtype)
                )
            )
    n_params = len(in_names)
    all_names = in_names + out_names
    if part_name is not None:
        all_names = all_names + [part_name]

    def _body(*args):
        operands = list(args)
        if part_name is not None:
            operands.append(bass2jax.partition_id_tensor())
        outs = bass2jax._bass_exec_p.bind(
            *operands,
            out_avals=tuple(out_avals),
            in_names=tuple(all_names),
            out_names=tuple(out_names),
            lowering_input_output_aliases=(),
            sim_require_finite=True,
            sim_require_nnan=True,
            nc=nc,
        )
        return tuple(outs)

    devices = jax.devices()[:NCORES]
    mesh = Mesh(np.asarray(devices), ("core",))
    nin = n_params + len(out_names)
    sharded = jax.jit(
        shard_map(
            _body,
            mesh=mesh,
            in_specs=(PartitionSpec("core"),) * nin,
            out_specs=(PartitionSpec("core"),) * len(out_names),
            check_rep=False,
        ),
        keep_unused=True,
    )

    zero_shapes = [(NCORES * a.shape[0], *a.shape[1:]) for a in out_avals]
    zero_dtypes = [a.dtype for a in out_avals]

    def run(in_maps):
        concat_in = [
            np.concatenate([np.asarray(m[name]) for m in in_maps], axis=0)
            for name in in_names
        ]
        zeros = [np.zeros(s, d) for s, d in zip(zero_shapes, zero_dtypes)]
        out_arrs = sharded(*concat_in, *zeros)
        jax.block_until_ready(out_arrs)
        res = [
            {
                name: np.asarray(out_arrs[i]).reshape(NCORES, *out_avals[i].shape)[c]
                for i, name in enumerate(out_names)
            }
            for c in range(NCORES)
        ]
        return res

    _cache["runner"] = run
    _cache["sharded"] = sharded
    _cache["meta"] = (in_names, out_names, out_avals)
    return run


def _host_prep(x, W_in, W_out, b_out, fusion_weights, routes):
    x = np.asarray(x, dtype=np.float32)
    W_in = np.asarray(W_in, dtype=np.float32)
    W_out = np.asarray(W_out, dtype=np.float32)
    b_out = np.asarray(b_out, dtype=np.float32)
    fw = np.asarray(fusion_weights, dtype=np.float32)
    rt = np.asarray(routes)

    Wc = (W_in @ W_out).astype(_bf16)
    xb16 = [np.ascontiguousarray(x[b].astype(_bf16)) for b in range(B)]
    # residual + bias, pre-transposed: [D, QROWS] fp32 per (b, q)
    xrb = [
        [
            np.ascontiguousarray(x[b, q * QROWS : (q + 1) * QROWS].T)
            + b_out[:, None]
            for q in range(4)
        ]
        for b in range(B)
    ]

    # densify A^T per seq-quarter: at_q[src, j] = sum of weights routing src -> (1024q + j)
    cols = np.repeat(np.arange(QROWS, dtype=np.int64), K)
    at_q = []
    for q in range(4):
        r = rt[q * QROWS : (q + 1) * QROWS].astype(np.int64).ravel()
        a = np.zeros((S, QROWS), np.float32)
        np.add.at(a, (r, cols), fw[q * QROWS : (q + 1) * QROWS].ravel())
        at_q.append(a.astype(_bf16))

    in_maps = []
    for c in range(NCORES):
        b, q = divmod(c, 4)
        in_maps.append(
            {"xb": xb16[b], "at": at_q[q], "wc": Wc, "xrb": xrb[b][q]}
        )
    return in_maps


def kernel(x, W_in, W_out, b_out, fusion_weights, routes):
    run = _get_runner()
    in_maps = _host_prep(x, W_in, W_out, b_out, fusion_weights, routes)
    res = run(in_maps)
    out = np.empty((B, S, D), np.float32)
    for c in range(NCORES):
        b, q = divmod(c, 4)
        out[b, q * QROWS : (q + 1) * QROWS] = res[c]["outT"].T
    return out


# revision 21
# speedup vs baseline: 12.6343x; 12.6343x over previous
"""CantorMultiheadFusion kernel for 8 Trainium2 NeuronCores.

Math: out = x + A @ x @ (W_in @ W_out) + b_out, where A is the (S,S) sparse
fusion matrix with A[s, routes[s,k]] += fusion_weights[s,k].

Strategy (per core): data-parallel over (batch b, seq quarter q); each core
computes 1024 output rows. The sparse gather-fuse runs as a dense matmul on
the PE array in transposed layout so the projection chains without any
on-device transposes:
  phase A: axT[d, s]  = sum_src x[src, d] * A^T[src, s]       (lhsT = x blocks)
  phase B: outT[d2, s] = sum_d Wc[d, d2] * axT[d, s] + (x^T + b_out)[d2, s]

The output is produced transposed ([D, rows] per core); the host reassembles
the (B, S, D) layout. On-device math is bf16 with fp32 PSUM accumulation; the
residual+bias tensor stays fp32. Host preprocessing is input repacking only:
densifying the routing tables into A^T, casting to bf16, transposing slices.
"""

import numpy as np
import ml_dtypes

B, S, D, K = 2, 4096, 512, 32
NCORES = 8
QROWS = S // 4  # rows per core = 1024
DBLK = D // 128  # 4
KBLK = S // 128  # 32

_bf16 = ml_dtypes.bfloat16

_cache = {}


def _build_module(nk=KBLK):
    import concourse.mybir as mybir
    import concourse.tile as tile
    from concourse import bacc

    f32 = mybir.dt.float32
    bf16 = mybir.dt.bfloat16

    nc = bacc.Bacc("TRN2", target_bir_lowering=True)

    xb = nc.dram_tensor("xb", [nk * 128, D], bf16, kind="ExternalInput")
    at = nc.dram_tensor("at", [nk * 128, QROWS], bf16, kind="ExternalInput")
    wc = nc.dram_tensor("wc", [D, D], bf16, kind="ExternalInput")
    xrb = nc.dram_tensor("xrb", [D, QROWS], f32, kind="ExternalInput")
    outT = nc.dram_tensor("outT", [D, QROWS], f32, kind="ExternalOutput")

    with tile.TileContext(nc) as tc:
        with (
            tc.tile_pool(name="const", bufs=1) as cpool,
            tc.tile_pool(name="work", bufs=3) as wpool,
            tc.tile_pool(name="psum", bufs=4, space="PSUM") as ppool,
        ):
            # --- streamed loads (xb/at first: phase A consumes them) ---------
            xb_sb = []  # packed x[b] row-block k: [128, D]
            at_sb = []  # packed A^T row-block k: [128, QROWS]
            for k in range(nk):
                t = cpool.tile([128, D], bf16, tag=f"xb{k}")
                nc.sync.dma_start(out=t, in_=xb[k * 128 : (k + 1) * 128, :])
                xb_sb.append(t)
                t = cpool.tile([128, QROWS], bf16, tag=f"at{k}")
                nc.scalar.dma_start(out=t, in_=at[k * 128 : (k + 1) * 128, :])
                at_sb.append(t)

            wc_sb = []  # Wc row-block d1: [128, D] (needed only in phase B)
            for d1 in range(DBLK):
                t = cpool.tile([128, D], bf16, tag=f"wc{d1}")
                nc.sync.dma_start(out=t, in_=wc[d1 * 128 : (d1 + 1) * 128, :])
                wc_sb.append(t)

            xrb_sb = []  # (x^T + b_out) block d2: [128, QROWS] fp32
            for d2 in range(DBLK):
                t = cpool.tile([128, QROWS], f32, tag=f"xrb{d2}")
                nc.sync.dma_start(out=t, in_=xrb[d2 * 128 : (d2 + 1) * 128, :])
                xrb_sb.append(t)

            # --- phase A: axT[d] = x-block-col-d ^T @ A^T --------------------
            # k outer / d inner: each at-tile is consumed 4x right after its
            # DMA lands, so the PE never waits on the A^T stream. d=0 is
            # front-loaded (d>0 lag by ~10 k-steps) so the PSUM->SBUF copy of
            # axT[0..2] and phase B's early chains hide under phase A.
            ps_a = [
                ppool.tile([128, QROWS], f32, tag="ps", name=f"ps_a{d}")
                for d in range(DBLK)
            ]

            def _mm_a(d, k):
                for h in range(2):
                    nc.tensor.matmul(
                        ps_a[d][:, h * 512 : (h + 1) * 512],
                        xb_sb[k][:, d * 128 : (d + 1) * 128],
                        at_sb[k][:, h * 512 : (h + 1) * 512],
                        start=(k == 0),
                        stop=(k == nk - 1),
                    )

            for k in range(nk):
                for d in range(DBLK):
                    _mm_a(d, k)
            axT = []
            for d in range(DBLK):
                t = wpool.tile([128, QROWS], bf16, tag=f"axT{d}")
                if d % 2 == 0:
                    nc.vector.tensor_copy(t, ps_a[d])
                else:
                    nc.scalar.activation(
                        t, ps_a[d], mybir.ActivationFunctionType.Copy
                    )
                axT.append(t)

            # --- phase B: outT[d2] = Wc-chain @ axT + (x^T + b_out) ----------
            for d2 in range(DBLK):
                ps_b = ppool.tile([128, QROWS], f32, tag="ps", name=f"ps_b{d2}")
                for d1 in range(DBLK):
                    for h in range(2):
                        nc.tensor.matmul(
                            ps_b[:, h * 512 : (h + 1) * 512],
                            wc_sb[d1][:, d2 * 128 : (d2 + 1) * 128],
                            axT[d1][:, h * 512 : (h + 1) * 512],
                            start=(d1 == 0),
                            stop=(d1 == DBLK - 1),
                        )
                o = wpool.tile([128, QROWS], f32, tag="osb", name=f"osb{d2}")
                nc.vector.tensor_tensor(o, ps_b, xrb_sb[d2], mybir.AluOpType.add)
                nc.sync.dma_start(out=outT[d2 * 128 : (d2 + 1) * 128, :], in_=o)

    nc.finalize()
    return nc


def _get_runner(nk=KBLK):
    """Compile once per nk; return a callable(in_maps) -> list_of_out_dicts."""
    key = ("runner", nk)
    if key in _cache:
        return _cache[key]

    import jax
    from jax.sharding import Mesh, PartitionSpec
    from jax.experimental.shard_map import shard_map
    from concourse import bass2jax
    import concourse.mybir as mybir

    bass2jax.install_neuronx_cc_hook()
    nc = _build_module(nk)

    part_name = nc.partition_id_tensor.name if nc.partition_id_tensor else None
    in_names = []
    out_names = []
    out_avals = []
    for alloc in nc.m.functions[0].allocations:
        if not isinstance(alloc, bass2jax.mybir.MemoryLocationSet):
            continue
        name = alloc.memorylocations[0].name
        if alloc.kind == "ExternalInput":
            if name != part_name:
                in_names.append(name)
        elif alloc.kind == "ExternalOutput":
            out_names.append(name)
            out_avals.append(
                jax.core.ShapedArray(
                    tuple(alloc.tensor_shape), mybir.dt.np(alloc.dtype)
                )
            )
    n_params = len(in_names)
    all_names = in_names + out_names
    if part_name is not None:
        all_names = all_names + [part_name]

    def _body(*args):
        operands = list(args)
        if part_name is not None:
            operands.append(bass2jax.partition_id_tensor())
        outs = bass2jax._bass_exec_p.bind(
            *operands,
            out_avals=tuple(out_avals),
            in_names=tuple(all_names),
            out_names=tuple(out_names),
            lowering_input_output_aliases=(),
            sim_require_finite=True,
            sim_require_nnan=True,
            nc=nc,
        )
        return tuple(outs)

    devices = jax.devices()[:NCORES]
    mesh = Mesh(np.asarray(devices), ("core",))
    nin = n_params + len(out_names)
    sharded = jax.jit(
        shard_map(
            _body,
            mesh=mesh,
            in_specs=(PartitionSpec("core"),) * nin,
            out_specs=(PartitionSpec("core"),) * len(out_names),
            check_rep=False,
        ),
        keep_unused=True,
    )

    zero_shapes = [(NCORES * a.shape[0], *a.shape[1:]) for a in out_avals]
    zero_dtypes = [a.dtype for a in out_avals]

    def run(in_maps):
        concat_in = [
            np.concatenate([np.asarray(m[name]) for m in in_maps], axis=0)
            for name in in_names
        ]
        zeros = [np.zeros(s, d) for s, d in zip(zero_shapes, zero_dtypes)]
        out_arrs = sharded(*concat_in, *zeros)
        jax.block_until_ready(out_arrs)
        res = [
            {
                name: np.asarray(out_arrs[i]).reshape(NCORES, *out_avals[i].shape)[c]
                for i, name in enumerate(out_names)
            }
            for c in range(NCORES)
        ]
        return res

    _cache[key] = run
    _cache[("sharded", nk)] = sharded
    _cache[("meta", nk)] = (in_names, out_names, out_avals)
    return run


def _host_prep(x, W_in, W_out, b_out, fusion_weights, routes):
    """Returns (nk, in_maps). Packs only the nonzero 128-row source blocks of
    A^T (and the matching x blocks) per core, padded to the max count nk."""
    x = np.asarray(x, dtype=np.float32)
    W_in = np.asarray(W_in, dtype=np.float32)
    W_out = np.asarray(W_out, dtype=np.float32)
    b_out = np.asarray(b_out, dtype=np.float32)
    fw = np.asarray(fusion_weights, dtype=np.float32)
    rt = np.asarray(routes)

    Wc = (W_in @ W_out).astype(_bf16)
    xb16 = [x[b].astype(_bf16) for b in range(B)]
    # residual + bias, pre-transposed: [D, QROWS] fp32 per (b, q)
    xrb = [
        [
            np.ascontiguousarray(x[b, q * QROWS : (q + 1) * QROWS].T)
            + b_out[:, None]
            for q in range(4)
        ]
        for b in range(B)
    ]

    # densify A^T per seq-quarter and find its nonzero source blocks
    cols = np.repeat(np.arange(QROWS, dtype=np.int64), K)
    at_q = []
    kset_q = []
    for q in range(4):
        r = rt[q * QROWS : (q + 1) * QROWS].astype(np.int64).ravel()
        a = np.zeros((S, QROWS), np.float32)
        np.add.at(a, (r, cols), fw[q * QROWS : (q + 1) * QROWS].ravel())
        blocks = a.reshape(KBLK, 128, QROWS)
        ks = [k for k in range(KBLK) if np.any(blocks[k])]
        if not ks:
            ks = [0]
        at_q.append(a.astype(_bf16))
        kset_q.append(ks)

    nk = max(len(ks) for ks in kset_q)

    in_maps = []
    for c in range(NCORES):
        b, q = divmod(c, 4)
        ks = kset_q[q]
        at_p = np.zeros((nk * 128, QROWS), _bf16)
        xb_p = np.zeros((nk * 128, D), _bf16)
        for i, k in enumerate(ks):
            at_p[i * 128 : (i + 1) * 128] = at_q[q][k * 128 : (k + 1) * 128]
            xb_p[i * 128 : (i + 1) * 128] = xb16[b][k * 128 : (k + 1) * 128]
        in_maps.append({"xb": xb_p, "at": at_p, "wc": Wc, "xrb": xrb[b][q]})
    return nk, in_maps


def kernel(x, W_in, W_out, b_out, fusion_weights, routes):
    nk, in_maps = _host_prep(x, W_in, W_out, b_out, fusion_weights, routes)
    run = _get_runner(nk)
    res = run(in_maps)
    out = np.empty((B, S, D), np.float32)
    for c in range(NCORES):
        b, q = divmod(c, 4)
        out[b, q * QROWS : (q + 1) * QROWS] = res[c]["outT"].T
    return out


# revision 39
# speedup vs baseline: 13.3817x; 1.0592x over previous
"""CantorMultiheadFusion kernel for 8 Trainium2 NeuronCores.

Math: out = x + A @ x @ (W_in @ W_out) + b_out, where A is the (S,S) sparse
fusion matrix with A[s, routes[s,k]] += fusion_weights[s,k].

Strategy (per core): data-parallel over (batch b, seq quarter q); each core
computes 1024 output rows. The sparse gather-fuse runs as a dense matmul on
the PE array in transposed layout so the projection chains without any
on-device transposes:
  phase A: axT[d, s]  = sum_src x[src, d] * A^T[src, s]       (lhsT = x blocks)
  phase B: outT[d2, s] = sum_d Wc[d, d2] * axT[d, s] + (x^T + b_out)[d2, s]

The output is produced transposed ([D, rows] per core); the host reassembles
the (B, S, D) layout. On-device math is bf16 with fp32 PSUM accumulation; the
residual+bias tensor stays fp32. Host preprocessing is input repacking only:
densifying the routing tables into A^T, casting to bf16, transposing slices.
"""

import numpy as np
import ml_dtypes

B, S, D, K = 2, 4096, 512, 32
NCORES = 8
QROWS = S // 4  # rows per core = 1024
DBLK = D // 128  # 4
KBLK = S // 128  # 32

_bf16 = ml_dtypes.bfloat16

_cache = {}


FUSED_NK_MAX = 8


def _build_module(nk=KBLK):
    """Two variants by nk:

    - fused (nk <= FUSED_NK_MAX): phase P projects the packed x blocks by Wc
      first (xc = x_sel @ Wc, cheap since only nk blocks), then a single
      accumulation phase A' computes outT = xc_sel^T-chain @ A^T. Phase P
      fills the startup hole while the A^T stream is still arriving, and
      there is no post-phase projection tail.
    - split (nk > FUSED_NK_MAX): big phase A (x^T-chain @ A^T) then a small
      projection phase B by Wc. Cheaper when nk is large because P would
      scale with nk while B is constant.
    """
    import concourse.mybir as mybir
    import concourse.tile as tile
    from concourse import bacc

    f32 = mybir.dt.float32
    bf16 = mybir.dt.bfloat16
    fused = nk <= FUSED_NK_MAX

    nc = bacc.Bacc("TRN2", target_bir_lowering=True)

    if fused:
        # packed x^T: [D, nk*128]; entry [d, i*128 + c] = x_block_i[c, d]
        xtp = nc.dram_tensor("xtp", [D, nk * 128], bf16, kind="ExternalInput")
    else:
        xb = nc.dram_tensor("xb", [nk * 128, D], bf16, kind="ExternalInput")
    at = nc.dram_tensor("at", [nk * 128, QROWS], bf16, kind="ExternalInput")
    wc = nc.dram_tensor("wc", [D, D], bf16, kind="ExternalInput")
    xrb = nc.dram_tensor("xrb", [D, QROWS], f32, kind="ExternalInput")
    outT = nc.dram_tensor("outT", [D, QROWS], f32, kind="ExternalOutput")

    with tile.TileContext(nc) as tc:
        with (
            tc.tile_pool(name="const", bufs=1) as cpool,
            tc.tile_pool(name="work", bufs=3) as wpool,
            tc.tile_pool(name="psum", bufs=8 if fused else 4, space="PSUM") as ppool,
        ):
            # --- streamed loads ---------------------------------------------
            if fused:
                wc_sb = []
                xtp_sb = []  # x^T tile per d1: [128, nk*128], block i at cols i*128
                for d1 in range(DBLK):
                    t = cpool.tile([128, D], bf16, tag=f"wc{d1}")
                    nc.gpsimd.dma_start(out=t, in_=wc[d1 * 128 : (d1 + 1) * 128, :])
                    wc_sb.append(t)
                    t = cpool.tile([128, nk * 128], bf16, tag=f"xtp{d1}")
                    nc.sync.dma_start(
                        out=t, in_=xtp[d1 * 128 : (d1 + 1) * 128, :]
                    )
                    xtp_sb.append(t)
            else:
                xb_sb = []  # packed x[b] row-block k: [128, D]
                for k in range(nk):
                    t = cpool.tile([128, D], bf16, tag=f"xb{k}")
                    nc.sync.dma_start(out=t, in_=xb[k * 128 : (k + 1) * 128, :])
                    xb_sb.append(t)

            at_sb = []  # packed A^T row-block k: [128, QROWS]
            for k in range(nk):
                t = cpool.tile([128, QROWS], bf16, tag=f"at{k}")
                nc.scalar.dma_start(out=t, in_=at[k * 128 : (k + 1) * 128, :])
                at_sb.append(t)

            if not fused:
                wc_sb = []
                for d1 in range(DBLK):
                    t = cpool.tile([128, D], bf16, tag=f"wc{d1}")
                    nc.sync.dma_start(out=t, in_=wc[d1 * 128 : (d1 + 1) * 128, :])
                    wc_sb.append(t)

            xrb_sb = []  # (x^T + b_out) block d2: [128, QROWS] fp32
            for d2 in range(DBLK):
                t = cpool.tile([128, QROWS], f32, tag=f"xrb{d2}")
                eng = nc.gpsimd if fused else nc.sync
                eng.dma_start(out=t, in_=xrb[d2 * 128 : (d2 + 1) * 128, :])
                xrb_sb.append(t)

            if fused:
                # --- phase P: xc[i] = x_block[i] @ Wc ------------------------
                # d1 outer: paced by the (xtp[d1], wc[d1]) tile arrivals, all
                # nk accumulation groups advance together.
                ps_p = [
                    ppool.tile([128, D], f32, tag="ps", name=f"ps_p{i}")
                    for i in range(nk)
                ]
                for d1 in range(DBLK):
                    for i in range(nk):
                        nc.tensor.matmul(
                            ps_p[i],
                            xtp_sb[d1][:, i * 128 : (i + 1) * 128],
                            wc_sb[d1],
                            start=(d1 == 0),
                            stop=(d1 == DBLK - 1),
                        )
                xc_sb = []
                for i in range(nk):
                    t = wpool.tile([128, D], bf16, tag=f"xc{i % 4}", name=f"xc{i}")
                    if i % 2 == 0:
                        nc.vector.tensor_copy(t, ps_p[i])
                    else:
                        nc.scalar.activation(
                            t, ps_p[i], mybir.ActivationFunctionType.Copy
                        )
                    xc_sb.append(t)

                # --- phase A': outT-psum[d2,h] = xc-chain @ A^T --------------
                # group outer: each (d2, h) output group finishes its whole
                # block chain early so its residual-add + store pipeline
                # behind the PE while later groups stream.
                for d2 in range(DBLK):
                    o = wpool.tile([128, QROWS], f32, tag="osb", name=f"osb{d2}")
                    for h in range(2):
                        hs = slice(h * 512, (h + 1) * 512)
                        ps_o = ppool.tile(
                            [128, 512], f32, tag="ps", name=f"ps_o{d2}_{h}"
                        )
                        for i in range(nk):
                            nc.tensor.matmul(
                                ps_o,
                                xc_sb[i][:, d2 * 128 : (d2 + 1) * 128],
                                at_sb[i][:, h * 512 : (h + 1) * 512],
                                start=(i == 0),
                                stop=(i == nk - 1),
                            )
                        nc.vector.tensor_tensor(
                            o[:, hs],
                            ps_o,
                            xrb_sb[d2][:, hs],
                            mybir.AluOpType.add,
                        )
                        ring = nc.sync if (d2 + h) % 2 == 0 else nc.scalar
                        ring.dma_start(
                            out=outT[d2 * 128 : (d2 + 1) * 128, hs], in_=o[:, hs]
                        )
            else:
                # --- phase A: axT[d] = x-block-col-d ^T @ A^T ----------------
                # k outer / d inner: each at-tile is consumed right after its
                # DMA lands, so the PE never waits on the A^T stream.
                ps_a = [
                    ppool.tile([128, QROWS], f32, tag="ps2", name=f"ps_a{d}")
                    for d in range(DBLK)
                ]
                for k in range(nk):
                    for d in range(DBLK):
                        for h in range(2):
                            nc.tensor.matmul(
                                ps_a[d][:, h * 512 : (h + 1) * 512],
                                xb_sb[k][:, d * 128 : (d + 1) * 128],
                                at_sb[k][:, h * 512 : (h + 1) * 512],
                                start=(k == 0),
                                stop=(k == nk - 1),
                            )
                axT = []
                for d in range(DBLK):
                    t = wpool.tile([128, QROWS], bf16, tag=f"axT{d}")
                    if d % 2 == 0:
                        nc.vector.tensor_copy(t, ps_a[d])
                    else:
                        nc.scalar.activation(
                            t, ps_a[d], mybir.ActivationFunctionType.Copy
                        )
                    axT.append(t)

                # --- phase B: outT[d2] = Wc-chain @ axT + (x^T + b_out) ------
                for d2 in range(DBLK):
                    ps_b = ppool.tile(
                        [128, QROWS], f32, tag="ps2", name=f"ps_b{d2}"
                    )
                    for d1 in range(DBLK):
                        for h in range(2):
                            nc.tensor.matmul(
                                ps_b[:, h * 512 : (h + 1) * 512],
                                wc_sb[d1][:, d2 * 128 : (d2 + 1) * 128],
                                axT[d1][:, h * 512 : (h + 1) * 512],
                                start=(d1 == 0),
                                stop=(d1 == DBLK - 1),
                            )
                    o = wpool.tile([128, QROWS], f32, tag="osb", name=f"osb{d2}")
                    for h in range(2):
                        hs = slice(h * 512, (h + 1) * 512)
                        nc.vector.tensor_tensor(
                            o[:, hs],
                            ps_b[:, hs],
                            xrb_sb[d2][:, hs],
                            mybir.AluOpType.add,
                        )
                        ring = nc.sync if (d2 + h) % 2 == 0 else nc.scalar
                        ring.dma_start(
                            out=outT[d2 * 128 : (d2 + 1) * 128, hs], in_=o[:, hs]
                        )

    nc.finalize()
    return nc


def _get_runner(nk=KBLK):
    """Compile once per nk; return a callable(in_maps) -> list_of_out_dicts."""
    key = ("runner", nk)
    if key in _cache:
        return _cache[key]

    import jax
    from jax.sharding import Mesh, PartitionSpec
    from jax.experimental.shard_map import shard_map
    from concourse import bass2jax
    import concourse.mybir as mybir

    bass2jax.install_neuronx_cc_hook()
    nc = _build_module(nk)

    part_name = nc.partition_id_tensor.name if nc.partition_id_tensor else None
    in_names = []
    out_names = []
    out_avals = []
    for alloc in nc.m.functions[0].allocations:
        if not isinstance(alloc, bass2jax.mybir.MemoryLocationSet):
            continue
        name = alloc.memorylocations[0].name
        if alloc.kind == "ExternalInput":
            if name != part_name:
                in_names.append(name)
        elif alloc.kind == "ExternalOutput":
            out_names.append(name)
            out_avals.append(
                jax.core.ShapedArray(
                    tuple(alloc.tensor_shape), mybir.dt.np(alloc.dtype)
                )
            )
    n_params = len(in_names)
    all_names = in_names + out_names
    if part_name is not None:
        all_names = all_names + [part_name]

    def _body(*args):
        operands = list(args)
        if part_name is not None:
            operands.append(bass2jax.partition_id_tensor())
        outs = bass2jax._bass_exec_p.bind(
            *operands,
            out_avals=tuple(out_avals),
            in_names=tuple(all_names),
            out_names=tuple(out_names),
            lowering_input_output_aliases=(),
            sim_require_finite=True,
            sim_require_nnan=True,
            nc=nc,
        )
        return tuple(outs)

    devices = jax.devices()[:NCORES]
    mesh = Mesh(np.asarray(devices), ("core",))
    nin = n_params + len(out_names)
    sharded = jax.jit(
        shard_map(
            _body,
            mesh=mesh,
            in_specs=(PartitionSpec("core"),) * nin,
            out_specs=(PartitionSpec("core"),) * len(out_names),
            check_rep=False,
        ),
        keep_unused=True,
    )

    zero_shapes = [(NCORES * a.shape[0], *a.shape[1:]) for a in out_avals]
    zero_dtypes = [a.dtype for a in out_avals]

    def run(in_maps):
        concat_in = [
            np.concatenate([np.asarray(m[name]) for m in in_maps], axis=0)
            for name in in_names
        ]
        zeros = [np.zeros(s, d) for s, d in zip(zero_shapes, zero_dtypes)]
        out_arrs = sharded(*concat_in, *zeros)
        jax.block_until_ready(out_arrs)
        res = [
            {
                name: np.asarray(out_arrs[i]).reshape(NCORES, *out_avals[i].shape)[c]
                for i, name in enumerate(out_names)
            }
            for c in range(NCORES)
        ]
        return res

    _cache[key] = run
    _cache[("sharded", nk)] = sharded
    _cache[("meta", nk)] = (in_names, out_names, out_avals)
    return run


def _host_prep(x, W_in, W_out, b_out, fusion_weights, routes):
    """Returns (nk, in_maps). Packs only the nonzero 128-row source blocks of
    A^T (and the matching x blocks) per core, padded to the max count nk."""
    x = np.asarray(x, dtype=np.float32)
    W_in = np.asarray(W_in, dtype=np.float32)
    W_out = np.asarray(W_out, dtype=np.float32)
    b_out = np.asarray(b_out, dtype=np.float32)
    fw = np.asarray(fusion_weights, dtype=np.float32)
    rt = np.asarray(routes)

    Wc = (W_in @ W_out).astype(_bf16)
    xb16 = [x[b].astype(_bf16) for b in range(B)]
    # residual + bias, pre-transposed: [D, QROWS] fp32 per (b, q)
    xrb = [
        [
            np.ascontiguousarray(x[b, q * QROWS : (q + 1) * QROWS].T)
            + b_out[:, None]
            for q in range(4)
        ]
        for b in range(B)
    ]

    # densify A^T per seq-quarter and find its nonzero source blocks
    cols = np.repeat(np.arange(QROWS, dtype=np.int64), K)
    at_q = []
    kset_q = []
    for q in range(4):
        r = rt[q * QROWS : (q + 1) * QROWS].astype(np.int64).ravel()
        a = np.zeros((S, QROWS), np.float32)
        np.add.at(a, (r, cols), fw[q * QROWS : (q + 1) * QROWS].ravel())
        blocks = a.reshape(KBLK, 128, QROWS)
        ks = [k for k in range(KBLK) if np.any(blocks[k])]
        if not ks:
            ks = [0]
        at_q.append(a.astype(_bf16))
        kset_q.append(ks)

    nk = max(len(ks) for ks in kset_q)

    fused = nk <= FUSED_NK_MAX
    in_maps = []
    for c in range(NCORES):
        b, q = divmod(c, 4)
        ks = kset_q[q]
        at_p = np.zeros((nk * 128, QROWS), _bf16)
        for i, k in enumerate(ks):
            at_p[i * 128 : (i + 1) * 128] = at_q[q][k * 128 : (k + 1) * 128]
        m = {"at": at_p, "wc": Wc, "xrb": xrb[b][q]}
        if fused:
            xtp = np.zeros((D, nk * 128), _bf16)
            for i, k in enumerate(ks):
                xtp[:, i * 128 : (i + 1) * 128] = xb16[b][
                    k * 128 : (k + 1) * 128
                ].T
            m["xtp"] = xtp
        else:
            xb_p = np.zeros((nk * 128, D), _bf16)
            for i, k in enumerate(ks):
                xb_p[i * 128 : (i + 1) * 128] = xb16[b][k * 128 : (k + 1) * 128]
            m["xb"] = xb_p
        in_maps.append(m)
    return nk, in_maps


def kernel(x, W_in, W_out, b_out, fusion_weights, routes):
    nk, in_maps = _host_prep(x, W_in, W_out, b_out, fusion_weights, routes)
    run = _get_runner(nk)
    res = run(in_maps)
    out = np.empty((B, S, D), np.float32)
    for c in range(NCORES):
        b, q = divmod(c, 4)
        out[b, q * QROWS : (q + 1) * QROWS] = res[c]["outT"].T
    return out


# revision 40
# speedup vs baseline: 14.0535x; 1.0502x over previous
"""CantorMultiheadFusion kernel for 8 Trainium2 NeuronCores.

Math: out = x + A @ x @ (W_in @ W_out) + b_out, where A is the (S,S) sparse
fusion matrix with A[s, routes[s,k]] += fusion_weights[s,k].

Strategy (per core): data-parallel over (batch b, seq quarter q); each core
computes 1024 output rows. The sparse gather-fuse runs as a dense matmul on
the PE array in transposed layout so the projection chains without any
on-device transposes:
  phase A: axT[d, s]  = sum_src x[src, d] * A^T[src, s]       (lhsT = x blocks)
  phase B: outT[d2, s] = sum_d Wc[d, d2] * axT[d, s] + (x^T + b_out)[d2, s]

The output is produced transposed ([D, rows] per core); the host reassembles
the (B, S, D) layout. On-device math is bf16 with fp32 PSUM accumulation; the
residual+bias tensor stays fp32. Host preprocessing is input repacking only:
densifying the routing tables into A^T, casting to bf16, transposing slices.
"""

import numpy as np
import ml_dtypes

B, S, D, K = 2, 4096, 512, 32
NCORES = 8
QROWS = S // 4  # rows per core = 1024
DBLK = D // 128  # 4
KBLK = S // 128  # 32

_bf16 = ml_dtypes.bfloat16

_cache = {}


FUSED_NK_MAX = 8


def _build_module(nk=KBLK):
    """Two variants by nk:

    - fused (nk <= FUSED_NK_MAX): phase P projects the packed x blocks by Wc
      first (xc = x_sel @ Wc, cheap since only nk blocks), then a single
      accumulation phase A' computes outT = xc_sel^T-chain @ A^T. Phase P
      fills the startup hole while the A^T stream is still arriving, and
      there is no post-phase projection tail.
    - split (nk > FUSED_NK_MAX): big phase A (x^T-chain @ A^T) then a small
      projection phase B by Wc. Cheaper when nk is large because P would
      scale with nk while B is constant.
    """
    import concourse.mybir as mybir
    import concourse.tile as tile
    from concourse import bacc

    f32 = mybir.dt.float32
    bf16 = mybir.dt.bfloat16
    fused = nk <= FUSED_NK_MAX

    nc = bacc.Bacc("TRN2", target_bir_lowering=True)

    if fused:
        # packed x^T: [D, nk*128]; entry [d, i*128 + c] = x_block_i[c, d]
        xtp = nc.dram_tensor("xtp", [D, nk * 128], bf16, kind="ExternalInput")
    else:
        xb = nc.dram_tensor("xb", [nk * 128, D], bf16, kind="ExternalInput")
    at = nc.dram_tensor("at", [nk * 128, QROWS], bf16, kind="ExternalInput")
    wc = nc.dram_tensor("wc", [D, D], bf16, kind="ExternalInput")
    xrb = nc.dram_tensor("xrb", [D, QROWS], f32, kind="ExternalInput")
    outT = nc.dram_tensor("outT", [D, QROWS], f32, kind="ExternalOutput")

    with tile.TileContext(nc) as tc:
        with (
            tc.tile_pool(name="const", bufs=1) as cpool,
            tc.tile_pool(name="work", bufs=3) as wpool,
            tc.tile_pool(name="psum", bufs=8 if fused else 4, space="PSUM") as ppool,
        ):
            # --- streamed loads ---------------------------------------------
            if fused:
                wc_sb = []
                xtp_sb = []  # x^T tile per d1: [128, nk*128], block i at cols i*128
                for d1 in range(DBLK):
                    t = cpool.tile([128, D], bf16, tag=f"wc{d1}")
                    nc.gpsimd.dma_start(out=t, in_=wc[d1 * 128 : (d1 + 1) * 128, :])
                    wc_sb.append(t)
                    t = cpool.tile([128, nk * 128], bf16, tag=f"xtp{d1}")
                    nc.sync.dma_start(
                        out=t, in_=xtp[d1 * 128 : (d1 + 1) * 128, :]
                    )
                    xtp_sb.append(t)
            else:
                xb_sb = []  # packed x[b] row-block k: [128, D]
                for k in range(nk):
                    t = cpool.tile([128, D], bf16, tag=f"xb{k}")
                    nc.sync.dma_start(out=t, in_=xb[k * 128 : (k + 1) * 128, :])
                    xb_sb.append(t)

            at_sb = []  # packed A^T row-block k: [128, QROWS]
            for k in range(nk):
                t = cpool.tile([128, QROWS], bf16, tag=f"at{k}")
                if fused:
                    # spread the stream over all three DMA queues so it has
                    # fully landed before phase A' consumes it back-to-back
                    eng = (nc.scalar, nc.scalar, nc.sync, nc.gpsimd)[k % 4]
                else:
                    eng = nc.scalar
                eng.dma_start(out=t, in_=at[k * 128 : (k + 1) * 128, :])
                at_sb.append(t)

            if not fused:
                wc_sb = []
                for d1 in range(DBLK):
                    t = cpool.tile([128, D], bf16, tag=f"wc{d1}")
                    nc.sync.dma_start(out=t, in_=wc[d1 * 128 : (d1 + 1) * 128, :])
                    wc_sb.append(t)

            xrb_sb = []  # (x^T + b_out) block d2: [128, QROWS] fp32
            for d2 in range(DBLK):
                t = cpool.tile([128, QROWS], f32, tag=f"xrb{d2}")
                eng = nc.gpsimd if fused else nc.sync
                eng.dma_start(out=t, in_=xrb[d2 * 128 : (d2 + 1) * 128, :])
                xrb_sb.append(t)

            if fused:
                # --- phase P: xc[i] = x_block[i] @ Wc ------------------------
                # d1 outer: paced by the (xtp[d1], wc[d1]) tile arrivals, all
                # nk accumulation groups advance together.
                ps_p = [
                    ppool.tile([128, D], f32, tag="ps", name=f"ps_p{i}")
                    for i in range(nk)
                ]
                for d1 in range(DBLK):
                    for i in range(nk):
                        nc.tensor.matmul(
                            ps_p[i],
                            xtp_sb[d1][:, i * 128 : (i + 1) * 128],
                            wc_sb[d1],
                            start=(d1 == 0),
                            stop=(d1 == DBLK - 1),
                        )
                xc_sb = []
                for i in range(nk):
                    t = wpool.tile([128, D], bf16, tag=f"xc{i % 4}", name=f"xc{i}")
                    if i % 2 == 0:
                        nc.vector.tensor_copy(t, ps_p[i])
                    else:
                        nc.scalar.activation(
                            t, ps_p[i], mybir.ActivationFunctionType.Copy
                        )
                    xc_sb.append(t)

                # --- phase A': outT-psum[d2,h] = xc-chain @ A^T --------------
                # group outer: each (d2, h) output group finishes its whole
                # block chain early so its residual-add + store pipeline
                # behind the PE while later groups stream.
                for d2 in range(DBLK):
                    o = wpool.tile([128, QROWS], f32, tag="osb", name=f"osb{d2}")
                    for h in range(2):
                        hs = slice(h * 512, (h + 1) * 512)
                        ps_o = ppool.tile(
                            [128, 512], f32, tag="ps", name=f"ps_o{d2}_{h}"
                        )
                        for i in range(nk):
                            nc.tensor.matmul(
                                ps_o,
                                xc_sb[i][:, d2 * 128 : (d2 + 1) * 128],
                                at_sb[i][:, h * 512 : (h + 1) * 512],
                                start=(i == 0),
                                stop=(i == nk - 1),
                            )
                        nc.vector.tensor_tensor(
                            o[:, hs],
                            ps_o,
                            xrb_sb[d2][:, hs],
                            mybir.AluOpType.add,
                        )
                        ring = nc.sync if (d2 + h) % 2 == 0 else nc.scalar
                        ring.dma_start(
                            out=outT[d2 * 128 : (d2 + 1) * 128, hs], in_=o[:, hs]
                        )
            else:
                # --- phase A: axT[d] = x-block-col-d ^T @ A^T ----------------
                # k outer / d inner: each at-tile is consumed right after its
                # DMA lands, so the PE never waits on the A^T stream.
                ps_a = [
                    ppool.tile([128, QROWS], f32, tag="ps2", name=f"ps_a{d}")
                    for d in range(DBLK)
                ]
                for k in range(nk):
                    for d in range(DBLK):
                        for h in range(2):
                            nc.tensor.matmul(
                                ps_a[d][:, h * 512 : (h + 1) * 512],
                                xb_sb[k][:, d * 128 : (d + 1) * 128],
                                at_sb[k][:, h * 512 : (h + 1) * 512],
                                start=(k == 0),
                                stop=(k == nk - 1),
                            )
                axT = []
                for d in range(DBLK):
                    t = wpool.tile([128, QROWS], bf16, tag=f"axT{d}")
                    if d % 2 == 0:
                        nc.vector.tensor_copy(t, ps_a[d])
                    else:
                        nc.scalar.activation(
                            t, ps_a[d], mybir.ActivationFunctionType.Copy
                        )
                    axT.append(t)

                # --- phase B: outT[d2] = Wc-chain @ axT + (x^T + b_out) ------
                for d2 in range(DBLK):
                    ps_b = ppool.tile(
                        [128, QROWS], f32, tag="ps2", name=f"ps_b{d2}"
                    )
                    for d1 in range(DBLK):
                        for h in range(2):
                            nc.tensor.matmul(
                                ps_b[:, h * 512 : (h + 1) * 512],
                                wc_sb[d1][:, d2 * 128 : (d2 + 1) * 128],
                                axT[d1][:, h * 512 : (h + 1) * 512],
                                start=(d1 == 0),
                                stop=(d1 == DBLK - 1),
                            )
                    o = wpool.tile([128, QROWS], f32, tag="osb", name=f"osb{d2}")
                    for h in range(2):
                        hs = slice(h * 512, (h + 1) * 512)
                        nc.vector.tensor_tensor(
                            o[:, hs],
                            ps_b[:, hs],
                            xrb_sb[d2][:, hs],
                            mybir.AluOpType.add,
                        )
                        ring = nc.sync if (d2 + h) % 2 == 0 else nc.scalar
                        ring.dma_start(
                            out=outT[d2 * 128 : (d2 + 1) * 128, hs], in_=o[:, hs]
                        )

    nc.finalize()
    return nc


def _get_runner(nk=KBLK):
    """Compile once per nk; return a callable(in_maps) -> list_of_out_dicts."""
    key = ("runner", nk)
    if key in _cache:
        return _cache[key]

    import jax
    from jax.sharding import Mesh, PartitionSpec
    from jax.experimental.shard_map import shard_map
    from concourse import bass2jax
    import concourse.mybir as mybir

    bass2jax.install_neuronx_cc_hook()
    nc = _build_module(nk)

    part_name = nc.partition_id_tensor.name if nc.partition_id_tensor else None
    in_names = []
    out_names = []
    out_avals = []
    for alloc in nc.m.functions[0].allocations:
        if not isinstance(alloc, bass2jax.mybir.MemoryLocationSet):
            continue
        name = alloc.memorylocations[0].name
        if alloc.kind == "ExternalInput":
            if name != part_name:
                in_names.append(name)
        elif alloc.kind == "ExternalOutput":
            out_names.append(name)
            out_avals.append(
                jax.core.ShapedArray(
                    tuple(alloc.tensor_shape), mybir.dt.np(alloc.dtype)
                )
            )
    n_params = len(in_names)
    all_names = in_names + out_names
    if part_name is not None:
        all_names = all_names + [part_name]

    def _body(*args):
        operands = list(args)
        if part_name is not None:
            operands.append(bass2jax.partition_id_tensor())
        outs = bass2jax._bass_exec_p.bind(
            *operands,
            out_avals=tuple(out_avals),
            in_names=tuple(all_names),
            out_names=tuple(out_names),
            lowering_input_output_aliases=(),
            sim_require_finite=True,
            sim_require_nnan=True,
            nc=nc,
        )
        return tuple(outs)

    devices = jax.devices()[:NCORES]
    mesh = Mesh(np.asarray(devices), ("core",))
    nin = n_params + len(out_names)
    sharded = jax.jit(
        shard_map(
            _body,
            mesh=mesh,
            in_specs=(PartitionSpec("core"),) * nin,
            out_specs=(PartitionSpec("core"),) * len(out_names),
            check_rep=False,
        ),
        keep_unused=True,
    )

    zero_shapes = [(NCORES * a.shape[0], *a.shape[1:]) for a in out_avals]
    zero_dtypes = [a.dtype for a in out_avals]

    def run(in_maps):
        concat_in = [
            np.concatenate([np.asarray(m[name]) for m in in_maps], axis=0)
            for name in in_names
        ]
        zeros = [np.zeros(s, d) for s, d in zip(zero_shapes, zero_dtypes)]
        out_arrs = sharded(*concat_in, *zeros)
        jax.block_until_ready(out_arrs)
        res = [
            {
                name: np.asarray(out_arrs[i]).reshape(NCORES, *out_avals[i].shape)[c]
                for i, name in enumerate(out_names)
            }
            for c in range(NCORES)
        ]
        return res

    _cache[key] = run
    _cache[("sharded", nk)] = sharded
    _cache[("meta", nk)] = (in_names, out_names, out_avals)
    return run


def _host_prep(x, W_in, W_out, b_out, fusion_weights, routes):
    """Returns (nk, in_maps). Packs only the nonzero 128-row source blocks of
    A^T (and the matching x blocks) per core, padded to the max count nk."""
    x = np.asarray(x, dtype=np.float32)
    W_in = np.asarray(W_in, dtype=np.float32)
    W_out = np.asarray(W_out, dtype=np.float32)
    b_out = np.asarray(b_out, dtype=np.float32)
    fw = np.asarray(fusion_weights, dtype=np.float32)
    rt = np.asarray(routes)

    Wc = (W_in @ W_out).astype(_bf16)
    xb16 = [x[b].astype(_bf16) for b in range(B)]
    # residual + bias, pre-transposed: [D, QROWS] fp32 per (b, q)
    xrb = [
        [
            np.ascontiguousarray(x[b, q * QROWS : (q + 1) * QROWS].T)
            + b_out[:, None]
            for q in range(4)
        ]
        for b in range(B)
    ]

    # densify A^T per seq-quarter and find its nonzero source blocks
    cols = np.repeat(np.arange(QROWS, dtype=np.int64), K)
    at_q = []
    kset_q = []
    for q in range(4):
        r = rt[q * QROWS : (q + 1) * QROWS].astype(np.int64).ravel()
        a = np.zeros((S, QROWS), np.float32)
        np.add.at(a, (r, cols), fw[q * QROWS : (q + 1) * QROWS].ravel())
        blocks = a.reshape(KBLK, 128, QROWS)
        ks = [k for k in range(KBLK) if np.any(blocks[k])]
        if not ks:
            ks = [0]
        at_q.append(a.astype(_bf16))
        kset_q.append(ks)

    nk = max(len(ks) for ks in kset_q)

    fused = nk <= FUSED_NK_MAX
    in_maps = []
    for c in range(NCORES):
        b, q = divmod(c, 4)
        ks = kset_q[q]
        at_p = np.zeros((nk * 128, QROWS), _bf16)
        for i, k in enumerate(ks):
            at_p[i * 128 : (i + 1) * 128] = at_q[q][k * 128 : (k + 1) * 128]
        m = {"at": at_p, "wc": Wc, "xrb": xrb[b][q]}
        if fused:
            xtp = np.zeros((D, nk * 128), _bf16)
            for i, k in enumerate(ks):
                xtp[:, i * 128 : (i + 1) * 128] = xb16[b][
                    k * 128 : (k + 1) * 128
                ].T
            m["xtp"] = xtp
        else:
            xb_p = np.zeros((nk * 128, D), _bf16)
            for i, k in enumerate(ks):
                xb_p[i * 128 : (i + 1) * 128] = xb16[b][k * 128 : (k + 1) * 128]
            m["xb"] = xb_p
        in_maps.append(m)
    return nk, in_maps


def kernel(x, W_in, W_out, b_out, fusion_weights, routes):
    nk, in_maps = _host_prep(x, W_in, W_out, b_out, fusion_weights, routes)
    run = _get_runner(nk)
    res = run(in_maps)
    out = np.empty((B, S, D), np.float32)
    for c in range(NCORES):
        b, q = divmod(c, 4)
        out[b, q * QROWS : (q + 1) * QROWS] = res[c]["outT"].T
    return out
